# revision 39
# baseline (speedup 1.0000x reference)
"""AngleAwareTripletLoss distributed Bass kernel for 8 TRN2 NeuronCores.

Collective-free design. Each core is fully independent:

  Host prep (numpy):
    - stable-sort rows by label; shard 512 rows/core.
    - normalize features; ship the transposed normalized table in
      fp8-e4m3 (x64), pre-packed in the exact device layout so every
      DMA is a plain 2D copy (5-D DMA patterns are expensive to issue).
    - angle-threshold operands in fp16 with hi/lo split (PE fp16
      multiplies are exact into fp32 PSUM, so asq is near-exact);
      per-core compressed label one-hots ride the SAME matmuls:
        maskish psum = SC*(900.25 - asq) - 16384*[same label]
        window  psum = asq + 32768*[same label]
    - row norms, |f|^2, valid masks, recon denominators.

  Device per core (SPMD, same graph, per-core input values):
    - pos-mining pre-pass for all 4 row blocks (window matmul, MAX8,
      FIND_INDEX8, feature/angle gathers) runs while the big tables
      stream in.
    - score: fp8 DoubleRow matmuls (K=256, 2 per 512-col chunk) per
      1024-col chunk into triple-buffered PSUM; ACT copies to fp16
      SBUF with scale 1/4096 -> score16 = sim.
    - one full-width fp16 tensor_tensor min (2x mode) -> msim;
      MAX8 + FIND_INDEX8 give the hardest-negative index.
    - triplet distances via |a-p|^2 = |a|^2+|p|^2-2 a.p with fused
      scalar_tensor_tensor accumulate passes; gather-consuming DVE
      work is deferred one row-block so gather latency never stalls
      the mining pipeline.
    - per-core partial sums [1,16] DMA'd out; host combines.
"""

import sys
from contextlib import ExitStack

for _p in ("/opt/trn_rl_repo",):
    if _p not in sys.path:
        sys.path.insert(0, _p)

import numpy as np
import ml_dtypes

import concourse.bass as bass
import concourse.mybir as mybir
from concourse.bass_utils import run_bass_kernel_spmd

B = 4096
F = 512
NCORES = 8
S = B // NCORES
P = 128
NB = S // P          # 4 row blocks per core
NJ = 4               # 1024-col score chunks
CW = 1024
MARGIN = 128
W = S + 2 * MARGIN   # 768 window
KA = 14              # maskish angle rows (hi/lo split operands)
KW = 13              # window angle rows (hi/lo split)
KT = 50              # total contraction rows (angle + label one-hots)
SC = 16.0            # maskish scale
QS = 64.0            # fp8 feature scale; psum carries QS^2 * sim
LBIG = 16384.0       # neg-side label exclusion, rides maskish matmul
WBIG = 32768.0       # pos-window same-label bias (> max asq 24300)

FP32 = mybir.dt.float32
FP16 = mybir.dt.float16
FP8 = mybir.dt.float8e4
U32 = mybir.dt.uint32
AF = mybir.ActivationFunctionType
OP = mybir.AluOpType
DR = mybir.MatmulPerfMode.DoubleRow
FP8NP = np.dtype(ml_dtypes.float8_e4m3fn)

SEM_ENGINE = {
    "dIn": "sync", "dInB": "sync", "dInC": "sync", "dIn2": "scalar",
    "dOut": "sync",
    "dGT0": "scalar", "dGT1": "scalar", "dGT2": "scalar", "dGT3": "scalar",
    "cPE": "tensor", "cACT": "scalar", "cDVE": "vector",
    "dP0": "gpsimd", "dP1": "gpsimd",
}
ASYNC_SEMS = {"dIn", "dInB", "dInC", "dIn2", "dOut",
              "dGT0", "dGT1", "dGT2", "dGT3", "dP0", "dP1"}


class Sched:
    """Single-wait-per-instruction scheduler with buffer dep tracking."""

    def __init__(self, nc, stack):
        self.nc = nc
        self.sems = {k: stack.enter_context(nc.semaphore(name=f'sem_{k}'))
                     for k in SEM_ENGINE}
        self.counts = {k: 0 for k in SEM_ENGINE}
        self.hw = {}      # (engine, sem) -> waited threshold
        self.bufw = {}    # buffer -> (sem, cnt) of last write
        self.bufr = {}    # buffer -> [(sem, cnt)] reads since last write

    def _needed(self, eng, deps):
        best = {}
        for d in deps:
            if d is None:
                continue
            s, c = d
            if c <= 0:
                continue
            if s in ASYNC_SEMS:
                # DMA completions are unordered within a sem; wait for
                # everything issued so far (sound: emission order is
                # topological, so earlier issues never gate on us).
                c = self.counts[s]
            if self.hw.get((eng, s), 0) >= c:
                continue
            best[s] = max(best.get(s, 0), c)
        return list(best.items())

    def run(self, sem, emit, *, n=1, reads=(), writes=(), extra=(),
            fuse=True):
        eng = SEM_ENGINE[sem]
        deps = []
        for b in reads:
            deps.append(self.bufw.get(b))
        for b in writes:
            deps.extend(self.bufr.get(b, []))
            deps.append(self.bufw.get(b))
        deps.extend(extra)
        gates = self._needed(eng, deps)
        if not fuse:
            for s, c in gates:
                getattr(self.nc, eng).wait_ge(self.sems[s], c)
                self.hw[(eng, s)] = c
            gates = []
        for s, c in gates[:-1]:
            getattr(self.nc, eng).wait_ge(self.sems[s], c)
            self.hw[(eng, s)] = c
        inst = emit()
        if gates:
            s, c = gates[-1]
            inst._wait_ge(self.sems[s], c)
            self.hw[(eng, s)] = c
        self.counts[sem] += n
        inst.then_inc(self.sems[sem], n)
        cur = (sem, self.counts[sem])
        for b in writes:
            self.bufw[b] = cur
            self.bufr[b] = []
        for b in reads:
            self.bufr.setdefault(b, []).append(cur)
        return cur


def build_graph():
    nc = bass.Bass(trn_type="TRN2", num_devices=NCORES)

    dp_ = nc.declare_dram_parameter
    featx = dp_("featx", [B, F + 4], FP32, isOutput=False)
    gq = dp_("gq", [NJ * P, 4096], FP8, isOutput=False)  # packed, 4 pieces
    ownq = dp_("ownq", [P, 2 * 2 * NB * P], FP8, isOutput=False)
    f_shard = dp_("f_shard", [S, F], FP32, isOutput=False)
    row_ang = dp_("row_ang", [S, 3], FP32, isOutput=False)
    cangd = dp_("cang", [KT, B], FP16, isOutput=False)
    wangd = dp_("wang", [KT, W], FP16, isOutput=False)
    la_d = dp_("la_ang", [KT, S], FP16, isOutput=False)
    lc_d = dp_("lc_ang", [KT, S], FP16, isOutput=False)
    ansqd = dp_("ansq", [P, NB], FP32, isOutput=False)
    vmaskd = dp_("vmask", [P, NB], FP32, isOutput=False)
    wsd = dp_("wsv", [1, 1], FP32, isOutput=False)
    onesPd = dp_("ones128", [P, 1], FP32, isOutput=False)
    out = dp_("out", [1, 16], FP32, isOutput=True)

    sb_ = nc.alloc_sbuf_tensor
    # big tables (already in device layout; plain 2D DMAs)
    GT8 = sb_("s_GT", [P, 2 * 2 * 8 * F], FP8).ap()   # [P, ch kb2 two c]
    LT8 = sb_("s_LT", [P, 2 * 2 * NB * P], FP8).ap()  # [P, kb2 two m q]
    cang = sb_("s_cang", [KT, B], FP16).ap()
    wang = sb_("s_wang", [KT, W], FP16).ap()
    la = sb_("s_la", [KT, S], FP16).ap()
    lc = sb_("s_lc", [KT, S], FP16).ap()
    fS = [sb_(f"s_fS{m}", [P, F], FP32).ap() for m in range(NB)]
    rang = [sb_(f"s_rang{m}", [P, 3], FP32).ap() for m in range(NB)]
    # mining buffers
    msim = [sb_(f"s_msim{t}", [P, B], FP16).ap() for t in range(2)]
    mkF = [sb_(f"s_mkF{t}", [P, B], FP16).ap() for t in range(2)]
    scF = [sb_(f"s_scF{t}", [P, B], FP16).ap() for t in range(2)]
    wsc = [sb_(f"s_wsc{t}", [P, W], FP32).ap() for t in range(2)]
    maxn8 = [sb_(f"s_maxn8{t}", [P, 8], FP16).ap() for t in range(2)]
    mtree = [sb_(f"s_mtree{t}", [P, B // 2], FP16).ap() for t in range(2)]
    idxn8 = [sb_(f"s_idxn8{t}", [P, 8], U32).ap() for t in range(2)]
    maxp8 = [sb_(f"s_maxp8{t}", [P, 8], FP32).ap() for t in range(2)]
    idxp8 = [sb_(f"s_idxp8{t}", [P, 8], U32).ap() for t in range(2)]
    posf = [sb_(f"s_posf{t}", [P, 1], FP32).ap() for t in range(2)]
    negf = [sb_(f"s_negf{t}", [P, 1], FP32).ap() for t in range(2)]
    posu = [sb_(f"s_posu{m}", [P, 1], U32).ap() for m in range(NB)]
    negu = [sb_(f"s_negu{t}", [P, 1], U32).ap() for t in range(2)]
    pFX = [sb_(f"s_pFX{m}", [P, F + 4], FP32).ap() for m in range(NB)]
    nFX = [sb_(f"s_nFX{t}", [P, F + 4], FP32).ap() for t in range(2)]
    d3 = [sb_(f"s_d3{t}", [P, 3], FP32).ap() for t in range(2)]
    scr = [sb_(f"s_scr{t}", [P, F], FP32).ap() for t in range(2)]
    apd = [sb_(f"s_apd{t}", [P, 1], FP32).ap() for t in range(2)]
    andt = [sb_(f"s_andt{t}", [P, 1], FP32).ap() for t in range(2)]
    t1b = [sb_(f"s_t1b{t}", [P, 1], FP32).ap() for t in range(2)]
    t2b = [sb_(f"s_t2b{t}", [P, 1], FP32).ap() for t in range(2)]
    # wide per-row-block accumulator tiles
    posq = sb_("s_posq", [P, NB], FP32).ap()
    negq = sb_("s_negq", [P, NB], FP32).ap()
    pasq = sb_("s_pasq", [P, NB], FP32).ap()
    nasq = sb_("s_nasq", [P, NB], FP32).ap()
    ansq = sb_("s_ansq", [P, NB], FP32).ap()
    vmask = sb_("s_vmask", [P, NB], FP32).ap()
    w1 = sb_("s_w1", [P, NB], FP32).ap()
    w2 = sb_("s_w2", [P, NB], FP32).ap()
    bq = sb_("s_bq", [P, NB], FP32).ap()
    wbq = sb_("s_wbq", [P, NB], FP32).ap()
    a_s1 = sb_("s_a_s1", [P, 4], FP32).ap()
    onesP = sb_("s_onesP", [P, 1], FP32).ap()
    wsB = sb_("s_wsB", [P, 1], FP32).ap()
    part_sb = sb_("s_part", [1, 16], FP32).ap()

    # PSUM: four 1024-col chunk buffers (all 8 banks); maskish and
    # score bursts alternate through them so the PE stream has long
    # wait-free runs (the tensor engine clock ramps 1.2->2.4 GHz only
    # under continuous execution).
    pC = [nc.alloc_psum_tensor(f"p_c{k}", [P, CW], FP32).ap()
          for k in range(4)]

    GT8v = GT8[:].rearrange("p (ch kb2 two c) -> p ch kb2 two c",
                            ch=8, kb2=2, two=2)
    LT8v = LT8[:].rearrange("p (kb2 two m q) -> p kb2 two m q",
                            kb2=2, two=2, m=NB)

    with ExitStack() as stack:
        sc = Sched(nc, stack)
        sy, ve, ac, te, gp = nc.sync, nc.vector, nc.scalar, nc.tensor, nc.gpsimd

        def dma(sem, dst, src, buf, reads=()):
            eng = SEM_ENGINE[sem]
            e = getattr(nc, eng)
            return sc.run(sem, lambda: e.dma_start(dst, src), n=16,
                          writes=(buf,), reads=reads)

        # ---------------- input DMAs ----------------
        # dIn: pos pre-pass operands (tiny, first)
        dma("dIn", lc[:], lc_d[:, :], "lc")
        dma("dIn", wang[:], wangd[:, :], "wang")
        dma("dIn", wsB[:], wsd[:1, :1].to_broadcast((P, 1)), "wsB")
        # dInB: score/maskish operands (cang before the big table so the
        # first maskish matmuls can start during the GT transfers)
        dma("dInB", LT8[:], ownq[:, :], "LT")
        dma("dInB", la[:], la_d[:, :], "la")
        dma("dInB", cang[:], cangd[:, :], "cang")
        # dGT: full fp8 table, 4 col-group pieces (plain 2D slices),
        # one semaphore per piece so chunk j only gates on piece j.
        for j in range(NJ):
            sc.run(f"dGT{j}", lambda j=j: ac.dma_start(
                GT8[:, j * 4096:(j + 1) * 4096],
                gq[j * P:(j + 1) * P, :]),
                n=16, writes=(f"GT{j}",))
        # dInC: row data (consumed by the deferred dot passes)
        for m in range(NB):
            dma("dInC", fS[m][:], f_shard[m * P:(m + 1) * P, :], f"fS{m}")
        for m in range(NB):
            dma("dInC", rang[m][:], row_ang[m * P:(m + 1) * P, :], f"rang{m}")
        dma("dInC", ansq[:], ansqd[:, :], "ansq")
        dma("dInC", vmask[:], vmaskd[:, :], "vmask")
        dma("dInC", onesP[:], onesPd[:, :], "onesP")

        # ---------------- pos-mining pre-pass (all blocks) ----------------
        for m in range(NB):
            t = m % 2
            pw = pC[m % 2]
            pwb = f"pC{m % 2}"
            sc.run("cPE", lambda m=m, pw=pw: te.matmul(
                pw[:, :F], lc[:, m * P:(m + 1) * P], wang[:, :F],
                start=True, stop=True),
                reads=("lc", "wang"), writes=(pwb,))
            sc.run("cPE", lambda m=m, pw=pw: te.matmul(
                pw[:, F:W], lc[:, m * P:(m + 1) * P], wang[:, F:W],
                start=True, stop=True),
                reads=("lc", "wang"), writes=(pwb,))
            sc.run("cACT", lambda t=t, pw=pw: ac.activation(
                wsc[t][:], pw[:, :W], AF.Copy),
                reads=(pwb,), writes=(f"wsc{t}",))
            sc.run("cDVE", lambda t=t: ve.max(out=maxp8[t][:], in_=wsc[t][:]),
                   reads=(f"wsc{t}",), writes=(f"maxp8{t}",))
            sc.run("cDVE", lambda t=t: ve.max_index(idxp8[t][:], maxp8[t][:],
                                                    wsc[t][:]),
                   reads=(f"wsc{t}", f"maxp8{t}"), writes=(f"idxp8{t}",))
            sc.run("cDVE", lambda t=t: ve.tensor_copy(posf[t][:],
                                                      idxp8[t][:, :1]),
                   reads=(f"idxp8{t}",), writes=(f"posf{t}",))
            sc.run("cDVE", lambda t=t: ve.tensor_scalar(
                posf[t][:], posf[t][:], wsB[:, :1], 0.0,
                op0=OP.add, op1=OP.max),
                reads=(f"posf{t}", "wsB"), writes=(f"posf{t}",))
            sc.run("cDVE", lambda t=t: ve.tensor_scalar(
                posf[t][:], posf[t][:], float(B - 1), None, op0=OP.min),
                reads=(f"posf{t}",), writes=(f"posf{t}",))
            sc.run("cDVE", lambda t=t, m=m: ve.tensor_copy(posu[m][:],
                                                           posf[t][:]),
                   reads=(f"posf{t}",), writes=(f"posu{m}",))
            dPm = f"dP{t}"
            sc.run(dPm, lambda m=m: gp.indirect_dma_start(
                pFX[m][:], None, featx[:, :],
                bass.IndirectOffsetOnAxis(ap=posu[m][:, :1], axis=0)),
                n=16, reads=(f"posu{m}",), writes=(f"pFX{m}",))

        # ---------------- per-row-block neg mining ----------------
        deferred = [None] * NB

        for m in range(NB):
            t = m % 2

            for j in range(NJ):
                # maskish matmul burst (fp16, K=KT)
                sc.run("cPE", lambda m=m, j=j: te.matmul(
                    pC[j][:, :F], la[:, m * P:(m + 1) * P],
                    cang[:, j * CW:j * CW + F], start=True, stop=True),
                    reads=("la", "cang"), writes=(f"pC{j}",))
                sc.run("cPE", lambda m=m, j=j: te.matmul(
                    pC[j][:, F:], la[:, m * P:(m + 1) * P],
                    cang[:, j * CW + F:(j + 1) * CW], start=True, stop=True),
                    reads=("la", "cang"), writes=(f"pC{j}",))
            for j in range(NJ):
                sc.run("cACT", lambda t=t, j=j: ac.activation(
                    mkF[t][:, j * CW:(j + 1) * CW], pC[j][:], AF.Copy),
                    reads=(f"pC{j}",), writes=(f"mkF{t}_{j}",))
            for j in range(NJ):
                # score burst: fp8 DoubleRow (K=256) x2 per 512-col half
                for h in range(2):
                    ch = 2 * j + h
                    for kb2 in range(2):
                        sc.run("cPE", lambda m=m, ch=ch, h=h, kb2=kb2, j=j:
                               te.matmul(
                                   pC[j][:, h * F:(h + 1) * F],
                                   LT8v[:, kb2, :, m, :],
                                   GT8v[:, ch, kb2, :, :],
                                   start=(kb2 == 0), stop=(kb2 == 1),
                                   perf_mode=DR),
                               reads=("LT", f"GT{ch // 2}"),
                               writes=(f"pC{j}",))
            for j in range(NJ):
                # score16 = sim
                sc.run("cACT", lambda t=t, j=j: ac.activation(
                    scF[t][:, j * CW:(j + 1) * CW], pC[j][:], AF.Copy,
                    scale=1.0 / (QS * QS)),
                    reads=(f"pC{j}",), writes=(f"scF{t}_{j}",))

            # ---- pos-side + recon math (pre-pass gathers long done) ----
            sc.run("cDVE", lambda t=t, m=m: ve.scalar_tensor_tensor(
                scr[t][:], fS[m][:], 1.0, pFX[m][:, :F],
                op0=OP.mult, op1=OP.mult, accum_out=apd[t][:]),
                reads=(f"fS{m}", f"pFX{m}"),
                writes=(f"scr{t}", f"apd{t}"))
            sc.run("cDVE", lambda t=t, m=m: ve.tensor_tensor(
                t1b[t][:], ansq[:, m:m + 1], pFX[m][:, F + 3:F + 4],
                op=OP.add),
                reads=("ansq", f"pFX{m}"), writes=(f"t1b{t}",))
            sc.run("cDVE", lambda t=t, m=m: ve.scalar_tensor_tensor(
                posq[:, m:m + 1], apd[t][:], -2.0, t1b[t][:],
                op0=OP.mult, op1=OP.add),
                reads=(f"apd{t}", f"t1b{t}"), writes=(f"posq{m}",))
            sc.run("cDVE", lambda t=t, m=m: ve.tensor_tensor(
                d3[t][:], rang[m][:], pFX[m][:, F:F + 3],
                op=OP.subtract),
                reads=(f"rang{m}", f"pFX{m}"), writes=(f"d3{t}",))
            sc.run("cDVE", lambda t=t, m=m: ve.scalar_tensor_tensor(
                d3[t][:], d3[t][:], 1.0, d3[t][:],
                op0=OP.mult, op1=OP.mult,
                accum_out=pasq[:, m:m + 1]),
                reads=(f"d3{t}",), writes=(f"d3{t}", f"pasq{m}",))

            # ---- deferred neg-side work of the PREVIOUS block ----
            if m > 0 and deferred[m - 1] is not None:
                deferred[m - 1]()
                deferred[m - 1] = None

            # ---- mask+select per chunk (starts as soon as each score
            # chunk drains, so the post-matmul serial tail is short) ----
            for j in range(NJ):
                sc.run("cDVE", lambda t=t, j=j: ve.tensor_tensor(
                    msim[t][:, j * CW:(j + 1) * CW],
                    mkF[t][:, j * CW:(j + 1) * CW],
                    scF[t][:, j * CW:(j + 1) * CW], op=OP.min),
                    reads=(f"mkF{t}_{j}", f"scF{t}_{j}"),
                    writes=(f"msim{t}_{j}",))
            # pairwise-max tree (fp16 TT runs 2x; MAX8 only runs 1x)
            MSIMALL = tuple(f"msim{t}_{j}" for j in range(NJ))
            sc.run("cDVE", lambda t=t: ve.tensor_tensor(
                mtree[t][:, :CW], msim[t][:, 0:CW], msim[t][:, CW:2 * CW],
                op=OP.max),
                reads=(f"msim{t}_0", f"msim{t}_1"), writes=(f"mtree{t}",))
            sc.run("cDVE", lambda t=t: ve.tensor_tensor(
                mtree[t][:, CW:2 * CW], msim[t][:, 2 * CW:3 * CW],
                msim[t][:, 3 * CW:4 * CW], op=OP.max),
                reads=(f"msim{t}_2", f"msim{t}_3"), writes=(f"mtree{t}",))
            for wdt in (1024, 512, 256):
                sc.run("cDVE", lambda t=t, wdt=wdt: ve.tensor_tensor(
                    mtree[t][:, :wdt], mtree[t][:, :wdt],
                    mtree[t][:, wdt:2 * wdt], op=OP.max),
                    reads=(f"mtree{t}",), writes=(f"mtree{t}",))
            sc.run("cDVE", lambda t=t: ve.max(out=maxn8[t][:],
                                              in_=mtree[t][:, :256]),
                   reads=(f"mtree{t}",), writes=(f"maxn8{t}",))
            sc.run("cDVE", lambda t=t: ve.max_index(idxn8[t][:], maxn8[t][:],
                                                    msim[t][:]),
                   reads=MSIMALL + (f"maxn8{t}",), writes=(f"idxn8{t}",))
            sc.run("cDVE", lambda t=t: ve.tensor_copy(negf[t][:],
                                                      idxn8[t][:, :1]),
                   reads=(f"idxn8{t}",), writes=(f"negf{t}",))
            sc.run("cDVE", lambda t=t: ve.tensor_scalar(
                negf[t][:], negf[t][:], 0.0, float(B - 1),
                op0=OP.max, op1=OP.min),
                reads=(f"negf{t}",), writes=(f"negf{t}",))
            sc.run("cDVE", lambda t=t: ve.tensor_copy(negu[t][:], negf[t][:]),
                   reads=(f"negf{t}",), writes=(f"negu{t}",))
            dPm = f"dP{t}"
            sc.run(dPm, lambda t=t: gp.indirect_dma_start(
                nFX[t][:], None, featx[:, :],
                bass.IndirectOffsetOnAxis(ap=negu[t][:, :1], axis=0)),
                n=16, reads=(f"negu{t}",), writes=(f"nFX{t}",))

            def make_deferred(m=m, t=t):
                def emit():
                    sc.run("cDVE", lambda: ve.scalar_tensor_tensor(
                        scr[t][:], fS[m][:], 1.0, nFX[t][:, :F],
                        op0=OP.mult, op1=OP.mult, accum_out=andt[t][:]),
                        reads=(f"fS{m}", f"nFX{t}"),
                        writes=(f"scr{t}", f"andt{t}"))
                    sc.run("cDVE", lambda: ve.tensor_tensor(
                        t2b[t][:], ansq[:, m:m + 1], nFX[t][:, F + 3:F + 4],
                        op=OP.add),
                        reads=("ansq", f"nFX{t}"), writes=(f"t2b{t}",))
                    sc.run("cDVE", lambda: ve.scalar_tensor_tensor(
                        negq[:, m:m + 1], andt[t][:], -2.0, t2b[t][:],
                        op0=OP.mult, op1=OP.add),
                        reads=(f"andt{t}", f"t2b{t}"), writes=(f"negq{m}",))
                    sc.run("cDVE", lambda: ve.tensor_tensor(
                        d3[t][:], rang[m][:], nFX[t][:, F:F + 3],
                        op=OP.subtract),
                        reads=(f"rang{m}", f"nFX{t}"), writes=(f"d3{t}",))
                    sc.run("cDVE", lambda: ve.scalar_tensor_tensor(
                        d3[t][:], d3[t][:], 1.0, d3[t][:],
                        op0=OP.mult, op1=OP.mult,
                        accum_out=nasq[:, m:m + 1]),
                        reads=(f"d3{t}",), writes=(f"d3{t}", f"nasq{m}",))
                return emit
            deferred[m] = make_deferred()

        deferred[NB - 1]()

        # ---------------- batched epilogue ----------------
        POSQ = tuple(f"posq{m}" for m in range(NB))
        NEGQ = tuple(f"negq{m}" for m in range(NB))
        PASQ = tuple(f"pasq{m}" for m in range(NB))
        NASQ = tuple(f"nasq{m}" for m in range(NB))
        sc.run("cDVE", lambda: ve.tensor_scalar_max(posq[:], posq[:], 0.0),
               reads=POSQ, writes=POSQ)
        sc.run("cDVE", lambda: ve.tensor_scalar_max(negq[:], negq[:], 0.0),
               reads=NEGQ, writes=NEGQ)
        sc.run("cACT", lambda: ac.activation(posq[:], posq[:], AF.Sqrt),
               reads=POSQ, writes=POSQ)
        sc.run("cACT", lambda: ac.activation(negq[:], negq[:], AF.Sqrt),
               reads=NEGQ, writes=NEGQ)
        sc.run("cDVE", lambda: ve.tensor_sub(bq[:], posq[:], negq[:]),
               reads=POSQ + NEGQ, writes=("bq",))
        sc.run("cDVE", lambda: ve.tensor_scalar(
            bq[:], bq[:], 0.2, 0.0, op0=OP.add, op1=OP.max),
            reads=("bq",), writes=("bq",))
        sc.run("cDVE", lambda: ve.tensor_scalar(
            w1[:], pasq[:], 2025.0, 1.0, op0=OP.is_gt, op1=OP.add),
            reads=PASQ, writes=("w1",))
        sc.run("cDVE", lambda: ve.tensor_scalar(
            w2[:], nasq[:], 225.0, None, op0=OP.is_lt),
            reads=NASQ, writes=("w2",))
        sc.run("cDVE", lambda: ve.tensor_scalar(
            w2[:], w2[:], 0.5, 1.0, op0=OP.mult, op1=OP.add),
            reads=("w2",), writes=("w2",))
        sc.run("cDVE", lambda: ve.tensor_tensor(
            w1[:], w1[:], w2[:], op=OP.mult),
            reads=("w1", "w2"), writes=("w1",))
        sc.run("cDVE", lambda: ve.tensor_tensor(
            wbq[:], w1[:], bq[:], op=OP.mult),
            reads=("w1", "bq"), writes=("wbq",))
        sc.run("cDVE", lambda: ve.tensor_tensor(
            wbq[:], wbq[:], vmask[:], op=OP.mult),
            reads=("wbq", "vmask"), writes=("wbq",))
        sc.run("cDVE", lambda: ve.tensor_reduce(
            a_s1[:, 0:1], wbq[:], axis=mybir.AxisListType.X, op=OP.add),
            reads=("wbq",), writes=("acc0",))
        sc.run("cDVE", lambda: ve.tensor_reduce(
            a_s1[:, 1:2], vmask[:], axis=mybir.AxisListType.X, op=OP.add),
            reads=("vmask",), writes=("acc1",))
        sc.run("cDVE", lambda: ve.memset(a_s1[:, 2:3], 0.0),
               writes=("acc2",))
        sc.run("cDVE", lambda: ve.memset(a_s1[:, 3:4], 0.0),
               writes=("acc3",))

        # partition reduce via PE; per-core partials out (host combines)
        sc.run("cPE", lambda: te.matmul(pC[0][:1, :4], onesP[:], a_s1[:],
                                        start=True, stop=True),
               reads=("onesP", "acc0", "acc1", "acc2", "acc3"),
               writes=("pC0",))
        sc.run("cDVE", lambda: ve.memset(part_sb[:], 0.0),
               writes=("part_sb",))
        sc.run("cACT", lambda: ac.activation(part_sb[:1, :4], pC[0][:1, :4],
                                             AF.Copy),
               reads=("pC0", "part_sb"), writes=("part_sb",))
        sc.run("dOut", lambda: sy.dma_start(out[:, :], part_sb[:]),
               n=16, reads=("part_sb",), writes=("out",))
        nc.sync.wait_ge(sc.sems["dOut"], sc.counts["dOut"])
        nc.all_engine_barrier()

    return nc


_cached = {}


def kernel(features, labels, angles, features_orig):
    features = np.ascontiguousarray(np.asarray(features, dtype=np.float32))
    angles = np.ascontiguousarray(np.asarray(angles, dtype=np.float32))
    features_orig = np.ascontiguousarray(np.asarray(features_orig, np.float32))
    labels = np.asarray(labels)

    perm = np.argsort(labels, kind="stable")
    fp = np.ascontiguousarray(features[perm])
    lp = labels[perm].astype(np.int64)
    ap_ = np.ascontiguousarray(angles[perm])
    fop = np.ascontiguousarray(features_orig[perm])

    counts = np.bincount(lp, minlength=256)
    assert counts.max() <= MARGIN

    # norms / normalized table, fp8 x64, packed in device layout:
    # gq[p, ch, kb2, two, c] = gn8[kb2*256+two*128+p, ch*512+c]
    nsq = (fp * fp).sum(1)
    nrm = np.sqrt(nsq)
    gn = fp / np.maximum(nrm, 1e-20)[:, None]
    g8 = (gn.T * QS).astype(FP8NP)                    # [F, B]
    gq = np.ascontiguousarray(
        g8.reshape(2, 2, P, 8, F).transpose(2, 3, 0, 1, 4).reshape(P, NJ, 4096)
        .transpose(1, 0, 2).reshape(NJ * P, 4096))
    fonsq = (fop * fop).sum(1)
    # recon term computed on host (elementwise; the mining stays on device)
    rnum = (fp * fop).sum(1)
    rden = np.maximum(nrm * np.sqrt(fonsq), 1e-8)
    recon_sum = float((rnum / rden).sum())

    has_pos = counts[lp] > 1
    has_neg = counts[lp] < B
    vm = (has_pos & has_neg).astype(np.float32)

    acol = ap_.astype(np.float32)
    acolsq = (acol ** 2).sum(1)

    # hi/lo split: PE fp16 multiplies are exact into fp32 PSUM, so
    # splitting each operand into fp16 hi + residual lo makes asq
    # near-exact (error ~ lo*lo, < 0.01) at no extra matmul cost.
    def hilo(x):
        h = x.astype(np.float16)
        l = (x.astype(np.float32) - h.astype(np.float32)).astype(np.float16)
        return h, l

    ah, al = hilo(acol)            # [B, 3] each
    sqh, sql = hilo(acolsq)        # [B] each

    featx = np.ascontiguousarray(np.concatenate(
        [fp, ap_, nsq[:, None]], axis=1).astype(np.float32))

    iota = np.arange(B)

    in_maps = []
    for c in range(NCORES):
        r0 = c * S
        rows = slice(r0, r0 + S)
        rah, ral = ah[rows], al[rows]
        rsqh, rsql = sqh[rows], sql[rows]

        # per-core compressed label one-hots
        labs_here = np.unique(lp[rows])
        nl = len(labs_here)
        assert KA + nl <= KT and KW + nl <= KT, f"core {c}: {nl} labels"
        lid = np.full(256, -1, np.int64)
        lid[labs_here] = np.arange(nl)

        # maskish operands (fp16):
        # psum = SC*(900.25 - asq(i,j)) - LBIG*[same label]
        cang = np.zeros((KT, B), np.float16)
        cang[0:3] = ah.T
        cang[3:6] = al.T
        cang[6:9] = ah.T
        cang[9] = SC
        cang[10] = SC
        cang[11] = sqh
        cang[12] = sql
        cang[13] = 1.0
        sel = lid[lp] >= 0
        cang[KA + lid[lp[sel]], iota[sel]] = 1.0
        la = np.zeros((KT, S), np.float16)
        la[0:3] = (2.0 * SC) * rah.T.astype(np.float32)
        la[3:6] = (2.0 * SC) * rah.T.astype(np.float32)
        la[6:9] = (2.0 * SC) * ral.T.astype(np.float32)
        la[9] = -rsqh
        la[10] = -rsql
        la[11] = -SC
        la[12] = -SC
        la[13] = SC * 900.25
        la[KA + lid[lp[rows]], np.arange(S)] = -LBIG

        # window (pos mining): psum = asq + WBIG*[same label]
        ws = r0 - MARGIN
        wi = ws + np.arange(W)
        ok = (wi >= 0) & (wi < B)
        wic = np.clip(wi, 0, B - 1)
        wang = np.zeros((KT, W), np.float16)
        wang[0:3] = np.where(ok, ah[wic].T, 0.0)
        wang[3:6] = np.where(ok, al[wic].T, 0.0)
        wang[6:9] = np.where(ok, ah[wic].T, 0.0)
        wang[9] = np.where(ok, 1.0, 0.0)
        wang[10] = np.where(ok, 1.0, 0.0)
        wang[11] = np.where(ok, sqh[wic], 0.0)
        wang[12] = np.where(ok, sql[wic], 0.0)
        wl = lid[lp[wic]]
        okw = ok & (wl >= 0)
        wang[KW + wl[okw], np.arange(W)[okw]] = 1.0
        lc_ = np.zeros((KT, S), np.float16)
        lc_[0:3] = -2.0 * rah.T.astype(np.float32)
        lc_[3:6] = -2.0 * rah.T.astype(np.float32)
        lc_[6:9] = -2.0 * ral.T.astype(np.float32)
        lc_[9] = rsqh
        lc_[10] = rsql
        lc_[11] = 1.0
        lc_[12] = 1.0
        lc_[KW + lid[lp[rows]], np.arange(S)] = WBIG

        def tile(v):
            return np.ascontiguousarray(
                v.astype(np.float32).reshape(NB, P).T)

        # ownq[p, kb2, two, m, q] = gn8[kb2*256+two*128+p, r0+m*128+q]
        o8 = g8[:, rows]                                # [F, S]
        ownq = np.ascontiguousarray(
            o8.reshape(2, 2, P, NB, P).transpose(2, 0, 1, 3, 4).reshape(P, -1))

        in_maps.append({
            "featx": featx,
            "gq": gq,
            "ownq": ownq,
            "f_shard": np.ascontiguousarray(fp[rows]),
            "row_ang": np.ascontiguousarray(ap_[rows]),
            "cang": cang,
            "wang": wang,
            "la_ang": la,
            "lc_ang": lc_,
            "ansq": tile(nsq[rows]),
            "vmask": tile(vm[rows]),
            "wsv": np.array([[float(ws)]], np.float32),
            "ones128": np.ones((P, 1), np.float32),
        })

    if "nc" not in _cached:
        _cached["nc"] = build_graph()
    res = run_bass_kernel_spmd(_cached["nc"], in_maps,
                               core_ids=list(range(NCORES)))
    wb = vs = 0.0
    for c in range(NCORES):
        o = np.asarray(res.results[c]["out"], dtype=np.float64).reshape(-1)
        wb += o[0]
        vs += o[1]
    t1 = wb / max(vs, 1.0)
    recon = 1.0 - recon_sum / B
    return np.float32(t1 + 0.1 * recon)


if __name__ == "__main__":
    pass
